# revision 29
# baseline (speedup 1.0000x reference)
"""DilateAttention (kernel=9, dilation=3, hd=32) on 8 NeuronCores via Bass/Tile.

Inputs  q,k,v: [4, 512, 1, 4096] f32  (B, d, 1, L); d = 16 heads x 32.
Output        [4, 1, 4096, 512] f32  (heads concatenated per token).

Math per (b, h): token n attends keys at n + 3*m - 12, m in 0..8 (zero-padded
outside [0, L)).  softmax over the 9 taps includes score-0 entries for
out-of-range taps (nn.Unfold zero-padding semantics).

Distribution: 64 (b,h) pairs -> 8 per core.  Host packs, per core, pairs of
heads into "kvs" [4, 128, L] (rows: k_i, k_j, v_i, v_j) and "q2" [4, 64, L]
(rows: q_i, q_j); pure slicing/stacking, all FLOPs happen on-device.

Per-core kernel (same SPMD program on all 8 cores, different data):
  Per head-pair: one DMA + one full-lane f32->bf16 cast for kv and for q.
  kvb [128, 4192] bf16 columns are shifted by HALO=12 with zero pads at both
  ends; qb [64, 4192] has zero pad columns at [L, W).
  For each 104-token tile t0 (key slab = positions [t0-12, t0+116)):
    mm1: S^T[u,t] = sum_d k[d, t0-12+u] * q[d, t0+t]      -> PSUM A
         (lhsT = kvb k-rows slab, rhs = qb block; 32-row contraction)
    mm2: vT[u,j]  = v[j, t0-12+u]                         -> PSUM B
         (lhsT = kvb v-rows slab, rhs = 32x32 identity)
    The two heads of a pair and mm1/mm2 sit on 4 distinct PE row-groups
    (partitions 0/32/64/96) so all four matmuls run concurrently.
    exp:  P^T = exp(S^T / sqrt(32))    ACT, PSUM->SBUF bf16
    band: P^T *= band01[u-t in {0,3,...,24}]   GPSIMD
    PV:   C[t, 0:32] = sum_u P^T[u,t] * vT[u,d]; a ones column appended to
          vT gives the softmax denominator in C[t, 32].
    normalize on DVE (reciprocal + broadcast multiply), DMA out.
  Tiles are processed in groups of G=4 sharing PSUM banks to amortize
  per-instruction overheads.
"""

import numpy as np

import concourse.bacc as bacc
import concourse.bass as bass
import concourse.mybir as mybir
from concourse.tile import TileContext

B, D, L = 4, 512, 4096
HD = 32
NHEAD = D // HD          # 16
NCORES = 8
BH_PER_CORE = (B * NHEAD) // NCORES   # 8
NPAIR = BH_PER_CORE // 2              # 4
HALO = 12                # dilation * (kernel-1) // 2
TSTEP = 104              # queries per tile = 128 - 2*HALO
SLAB = 128               # keys per tile
NT = (L + TSTEP - 1) // TSTEP         # 40 tiles per (b,h)
G = 4                    # tiles per PSUM group
NG = NT // G             # 10 groups
W = 4192                 # padded SBUF width (12 + 4096 + 12, rounded up)
SCALE = float(HD) ** -0.5

F32 = mybir.dt.float32
BF16 = mybir.dt.bfloat16


def _band01_np():
    # band01[u, t] = 1 iff key (t0-12+u) is a tap of query (t0+t):
    # u-t in {0, 3, ..., 24}.  t runs over the TSTEP valid queries per tile;
    # tiled Gx along t for the grouped/packed layout.
    u = np.arange(128)[:, None]
    t = np.arange(TSTEP)[None, :]
    d = u - t
    b = ((d >= 0) & (d <= 24) & (d % 3 == 0)).astype(np.float32)
    return np.tile(b, (1, G))


def _build_program(npair=NPAIR, ngroups=NG):
    import ml_dtypes

    nc = bacc.Bacc(None, target_bir_lowering=False)
    kvs = nc.dram_tensor("kvs", [NPAIR, 128, L], F32, kind="ExternalInput")
    q2 = nc.dram_tensor("q2", [NPAIR, 64, L], F32, kind="ExternalInput")
    out = nc.dram_tensor("out", [NPAIR, NG, TSTEP, 2 * G * HD], F32, kind="ExternalOutput")

    band01_dram = nc.inline_tensor(
        _band01_np().astype(ml_dtypes.bfloat16), name="band01"
    )
    # identities selecting the v rows (kvb rows 64-95 for head i, 96-127 for j)
    i0 = np.zeros((128, 32), dtype=ml_dtypes.bfloat16)
    for j in range(32):
        i0[64 + j, j] = 1.0
        i0[96 + j, j] = 1.0
    i0_dram = nc.inline_tensor(i0, name="i0ext")

    with TileContext(nc) as tc:
        from contextlib import ExitStack

        with ExitStack() as ctx:
            # ---- persistent tiles from a bufs=1 pool (ping-pong pairs) ----
            persist = ctx.enter_context(tc.tile_pool(name="persist", bufs=1))
            NSET = 3
            kvf = [
                persist.tile([128, L], F32, name=f"kvf{s}", tag=f"kvf{s}")
                for s in range(NSET)
            ]
            qf = [
                persist.tile([64, L], F32, name=f"qf{s}", tag=f"qf{s}")
                for s in range(NSET)
            ]
            kvb = [
                persist.tile([128, W], BF16, name=f"kvb{s}", tag=f"kvb{s}")
                for s in range(NSET)
            ]
            qb = [
                persist.tile([64, W], BF16, name=f"qb{s}", tag=f"qb{s}")
                for s in range(NSET)
            ]
            band01_sb = persist.tile([128, TSTEP * G], BF16, name="band01_sb", tag="band01_sb")
            i0_sb = persist.tile([128, 32], BF16, name="i0_sb", tag="i0_sb")

            nc.sync.dma_start(band01_sb[:, :], band01_dram[:, :])
            nc.sync.dma_start(i0_sb[:, :], i0_dram[:, :])

            # one-time zero inits for pad columns
            for s in range(NSET):
                nc.gpsimd.memset(kvb[s][:, 0:HALO], 0.0)
                nc.gpsimd.memset(kvb[s][:, HALO + L : W], 0.0)
                nc.gpsimd.memset(qb[s][:, L:W], 0.0)

            # ---- pools ----
            psA0 = ctx.enter_context(tc.tile_pool(name="psA0", bufs=2, space="PSUM"))
            psA1 = ctx.enter_context(tc.tile_pool(name="psA1", bufs=2, space="PSUM"))
            psB0 = ctx.enter_context(tc.tile_pool(name="psB0", bufs=1, space="PSUM"))
            psB1 = ctx.enter_context(tc.tile_pool(name="psB1", bufs=1, space="PSUM"))
            psC = ctx.enter_context(tc.tile_pool(name="psC", bufs=2, space="PSUM"))
            spP = ctx.enter_context(tc.tile_pool(name="spP", bufs=4))
            spV = ctx.enter_context(tc.tile_pool(name="spV", bufs=3))
            spR = ctx.enter_context(tc.tile_pool(name="spR", bufs=3))
            spS = ctx.enter_context(tc.tile_pool(name="spS", bufs=3))

            for pair in range(npair):
                s = pair % NSET
                # one DMA + one cast each for kv and q
                nc.sync.dma_start(kvf[s][:, 0 : L // 2], kvs[pair, :, 0 : L // 2])
                nc.sync.dma_start(qf[s][:, 0 : L // 2], q2[pair, :, 0 : L // 2])
                nc.sync.dma_start(kvf[s][:, L // 2 : L], kvs[pair, :, L // 2 : L])
                nc.sync.dma_start(qf[s][:, L // 2 : L], q2[pair, :, L // 2 : L])
                H4 = L // 4
                for ci in range(4):
                    c0, c1 = ci * H4, (ci + 1) * H4
                    nc.gpsimd.tensor_copy(
                        kvb[s][:, HALO + c0 : HALO + c1], kvf[s][:, c0:c1]
                    )
                    nc.gpsimd.tensor_copy(qb[s][:, c0:c1], qf[s][:, c0:c1])
                # standalone weight loads let PE observe the cast completion
                # here, keeping later matmuls at <=2 sync waits (ISA limit)
                nc.tensor.ldweights(kvb[s][0:32, 0:128])
                nc.tensor.ldweights(qb[s][0:32, 0:128])

                for g in range(ngroups):
                    A = [
                        psA0.tile([128, TSTEP * G], F32, name="A0", tag="A0"),
                        psA1.tile([128, TSTEP * G], F32, name="A1", tag="A1"),
                    ]
                    Bp = [
                        psB0.tile([128, 32 * G], F32, name="B0", tag="B0"),
                        psB1.tile([128, 32 * G], F32, name="B1", tag="B1"),
                    ]
                    Cp = psC.tile([128, 66 * G], F32, name="Cp")
                    for l in range(G):
                        t0 = (G * g + l) * TSTEP
                        for bh in range(2):
                            kbase = 32 * bh       # k rows of this head
                            vbase = 64 + 32 * bh  # v rows of this head
                            nc.tensor.matmul(
                                A[bh][:, TSTEP * l : TSTEP * (l + 1)],
                                kvb[s][kbase : kbase + 32, t0 : t0 + SLAB],
                                qb[s][kbase : kbase + 32, t0 : t0 + TSTEP],
                                start=True,
                                stop=True,
                                tile_position=(kbase, 0),
                            )
                            nc.tensor.matmul(
                                Bp[bh][:, 32 * l : 32 * (l + 1)],
                                kvb[s][vbase : vbase + 32, t0 : t0 + SLAB],
                                i0_sb[vbase : vbase + 32, :],
                                start=True,
                                stop=True,
                                tile_position=(vbase, 0),
                            )
                    P = [
                        spP.tile([128, TSTEP * G], BF16, name="P0", tag="P0"),
                        spP.tile([128, TSTEP * G], BF16, name="P1", tag="P1"),
                    ]
                    for bh in range(2):
                        nc.scalar.activation(
                            P[bh][:, :],
                            A[bh][:, :],
                            mybir.ActivationFunctionType.Exp,
                            bias=0.0,
                            scale=SCALE,
                        )
                        nc.vector.tensor_mul(P[bh][:, :], P[bh][:, :], band01_sb[:, :])
                    # vT: [128, 2G slots of (32 v-cols + 1 ones-col)]
                    vT = spV.tile([128, 33 * 2 * G], BF16, name="vT")
                    nc.gpsimd.memset(vT[:, 32 :: 33], 1.0)
                    vt3 = vT[:, :].rearrange("p (s d) -> p s d", s=2 * G)[:, :, 0:32]
                    for bh in range(2):
                        nc.vector.tensor_copy(
                            vt3[:, G * bh : G * (bh + 1), :],
                            Bp[bh][:, :].rearrange("p (s d) -> p s d", s=G),
                        )
                    for bh in range(2):
                        for l in range(G):
                            slot = G * bh + l
                            nc.tensor.matmul(
                                Cp[0:TSTEP, 33 * slot : 33 * (slot + 1)],
                                P[bh][:, TSTEP * l : TSTEP * (l + 1)],
                                vT[:, 33 * slot : 33 * (slot + 1)],
                                start=True,
                                stop=True,
                            )
                    r = spR.tile([128, 2 * G], F32, name="r")
                    nc.vector.reciprocal(r[0:TSTEP, :], Cp[0:TSTEP, 32 :: 33])
                    stage = spS.tile([128, 32 * 2 * G], F32, name="stage")
                    st_ap = stage[0:TSTEP, :].rearrange("p (s d) -> p s d", s=2 * G)
                    c_ap = Cp[0:TSTEP, :].rearrange("p (s d) -> p s d", s=2 * G)[:, :, 0:32]
                    r_b = r[0:TSTEP, :]
                    r_ap = bass.AP(
                        tensor=r_b.tensor,
                        offset=r_b.offset,
                        ap=[r_b.ap[0], [1, 2 * G], [0, 32]],
                    )
                    nc.vector.tensor_tensor(st_ap, c_ap, r_ap, op=mybir.AluOpType.mult)
                    # one contiguous output DMA per group (host reassembles)
                    nc.sync.dma_start(
                        out[pair, g, :, :], stage[0:TSTEP, :]
                    )
    nc.finalize()
    return nc


_CACHE = {}


def _get_program():
    if "nc" not in _CACHE:
        _CACHE["nc"] = _build_program()
    return _CACHE["nc"]


def make_in_maps(q, k, v):
    """Shard + pack FULL inputs into per-core input maps (host-side data
    movement only)."""
    q = np.ascontiguousarray(np.asarray(q), dtype=np.float32)
    k = np.ascontiguousarray(np.asarray(k), dtype=np.float32)
    v = np.ascontiguousarray(np.asarray(v), dtype=np.float32)
    qr = q.reshape(B * NHEAD, HD, L)
    kr = k.reshape(B * NHEAD, HD, L)
    vr = v.reshape(B * NHEAD, HD, L)

    in_maps = []
    for c in range(NCORES):
        base = c * BH_PER_CORE
        kvs = np.empty((NPAIR, 128, L), dtype=np.float32)
        q2 = np.empty((NPAIR, 64, L), dtype=np.float32)
        for p in range(NPAIR):
            i, j = base + 2 * p, base + 2 * p + 1
            kvs[p, 0:32] = kr[i]
            kvs[p, 32:64] = kr[j]
            kvs[p, 64:96] = vr[i]
            kvs[p, 96:128] = vr[j]
            q2[p, 0:32] = qr[i]
            q2[p, 32:64] = qr[j]
        in_maps.append({"kvs": kvs, "q2": q2})
    return in_maps


def assemble_output(results):
    """results: list of 8 per-core dicts with 'out'
    [NPAIR, NG, TSTEP, 2*G*HD]; scratch[p, g, t, (b, l, d)] holds token
    (G*g + l)*TSTEP + t of head-pair p, head b."""
    full = np.empty((B * NHEAD, L, HD), dtype=np.float32)
    for c in range(NCORES):
        sc = results[c]["out"].reshape(NPAIR, NG, TSTEP, 2, G, HD)
        # -> [pair, b, g, l, t, d] -> [bh, pos, d]
        sc = sc.transpose(0, 3, 1, 4, 2, 5).reshape(2 * NPAIR, NG * G * TSTEP, HD)
        full[c * BH_PER_CORE : (c + 1) * BH_PER_CORE] = sc[:, :L, :]
    full = full.reshape(B, NHEAD, L, HD).transpose(0, 2, 1, 3).reshape(B, L, D)
    return full.reshape(B, 1, L, D)


def kernel(q, k, v):
    from concourse.bass_utils import run_bass_kernel_spmd

    in_maps = make_in_maps(q, k, v)
    nc = _get_program()
    res = run_bass_kernel_spmd(nc, in_maps, core_ids=list(range(NCORES)))
    return assemble_output(res.results)


if __name__ == "__main__":
    rng = np.random.default_rng(0)
    q = rng.standard_normal((B, D, 1, L), dtype=np.float32)
    k = rng.standard_normal((B, D, 1, L), dtype=np.float32)
    v = rng.standard_normal((B, D, 1, L), dtype=np.float32)
    o = kernel(q=q, k=k, v=v)
    print("out", o.shape, o.dtype, float(np.abs(o).max()))


# revision 33
# speedup vs baseline: 1.0298x; 1.0298x over previous
"""DilateAttention (kernel=9, dilation=3, hd=32) on 8 NeuronCores via Bass/Tile.

Inputs  q,k,v: [4, 512, 1, 4096] f32  (B, d, 1, L); d = 16 heads x 32.
Output        [4, 1, 4096, 512] f32  (heads concatenated per token).

Math per (b, h): token n attends keys at n + 3*m - 12, m in 0..8 (zero-padded
outside [0, L)).  softmax over the 9 taps includes score-0 entries for
out-of-range taps (nn.Unfold zero-padding semantics).

Distribution: 64 (b,h) pairs -> 8 per core.  Host packs, per core, pairs of
heads into "kvs" [4, 128, L] (rows: k_i, k_j, v_i, v_j) and "q2" [4, 64, L]
(rows: q_i, q_j); pure slicing/stacking, all FLOPs happen on-device.

Per-core kernel (same SPMD program on all 8 cores, different data):
  Per head-pair: one DMA + one full-lane f32->bf16 cast for kv and for q.
  kvb [128, 4192] bf16 columns are shifted by HALO=12 with zero pads at both
  ends; qb [64, 4192] has zero pad columns at [L, W).
  For each 104-token tile t0 (key slab = positions [t0-12, t0+116)):
    mm1: S^T[u,t] = sum_d k[d, t0-12+u] * q[d, t0+t]      -> PSUM A
         (lhsT = kvb k-rows slab, rhs = qb block; 32-row contraction)
    mm2: vT[u,j]  = v[j, t0-12+u]                         -> PSUM B
         (lhsT = kvb v-rows slab, rhs = 32x32 identity)
    The two heads of a pair and mm1/mm2 sit on 4 distinct PE row-groups
    (partitions 0/32/64/96) so all four matmuls run concurrently.
    exp:  P^T = exp(S^T / sqrt(32))    ACT, PSUM->SBUF bf16
    band: P^T *= band01[u-t in {0,3,...,24}]   GPSIMD
    PV:   C[t, 0:32] = sum_u P^T[u,t] * vT[u,d]; a ones column appended to
          vT gives the softmax denominator in C[t, 32].
    normalize on DVE (reciprocal + broadcast multiply), DMA out.
  Tiles are processed in groups of G=4 sharing PSUM banks to amortize
  per-instruction overheads.
"""

import numpy as np

import concourse.bacc as bacc
import concourse.bass as bass
import concourse.mybir as mybir
from concourse.tile import TileContext

B, D, L = 4, 512, 4096
HD = 32
NHEAD = D // HD          # 16
NCORES = 8
BH_PER_CORE = (B * NHEAD) // NCORES   # 8
NPAIR = BH_PER_CORE // 2              # 4
HALO = 12                # dilation * (kernel-1) // 2
TSTEP = 104              # queries per tile = 128 - 2*HALO
SLAB = 128               # keys per tile
NT = (L + TSTEP - 1) // TSTEP         # 40 tiles per (b,h)
G = 4                    # tiles per PSUM group
NG = NT // G             # 10 groups
W = 4192                 # padded SBUF width (12 + 4096 + 12, rounded up)
SCALE = float(HD) ** -0.5

F32 = mybir.dt.float32
BF16 = mybir.dt.bfloat16


def _band01_np():
    # band01[u, t] = 1 iff key (t0-12+u) is a tap of query (t0+t):
    # u-t in {0, 3, ..., 24}.  t runs over the TSTEP valid queries per tile;
    # tiled Gx along t for the grouped/packed layout.
    u = np.arange(128)[:, None]
    t = np.arange(TSTEP)[None, :]
    d = u - t
    b = ((d >= 0) & (d <= 24) & (d % 3 == 0)).astype(np.float32)
    return np.tile(b, (1, G))


def _build_program(npair=NPAIR, ngroups=NG):
    import ml_dtypes

    nc = bacc.Bacc(None, target_bir_lowering=False)
    kvs = nc.dram_tensor("kvs", [NPAIR, 128, L], F32, kind="ExternalInput")
    q2 = nc.dram_tensor("q2", [NPAIR, 64, L], F32, kind="ExternalInput")
    out = nc.dram_tensor("out", [NPAIR, NG, TSTEP, 2 * G * HD], F32, kind="ExternalOutput")

    band01_dram = nc.inline_tensor(
        _band01_np().astype(ml_dtypes.bfloat16), name="band01"
    )
    # identities selecting the v rows (kvb rows 64-95 for head i, 96-127 for j)
    i0 = np.zeros((128, 32), dtype=ml_dtypes.bfloat16)
    for j in range(32):
        i0[64 + j, j] = 1.0
        i0[96 + j, j] = 1.0
    i0_dram = nc.inline_tensor(i0, name="i0ext")

    with TileContext(nc) as tc:
        from contextlib import ExitStack

        with ExitStack() as ctx:
            # ---- persistent tiles from a bufs=1 pool (ping-pong pairs) ----
            persist = ctx.enter_context(tc.tile_pool(name="persist", bufs=1))
            NSET = 3
            kvf = [
                persist.tile([128, L], F32, name=f"kvf{s}", tag=f"kvf{s}")
                for s in range(NSET)
            ]
            qf = [
                persist.tile([64, L], F32, name=f"qf{s}", tag=f"qf{s}")
                for s in range(NSET)
            ]
            kvb = [
                persist.tile([128, W], BF16, name=f"kvb{s}", tag=f"kvb{s}")
                for s in range(NSET)
            ]
            qb = [
                persist.tile([64, W], BF16, name=f"qb{s}", tag=f"qb{s}")
                for s in range(NSET)
            ]
            band01_sb = persist.tile([128, TSTEP * G], BF16, name="band01_sb", tag="band01_sb")
            i0_sb = persist.tile([128, 32], BF16, name="i0_sb", tag="i0_sb")

            nc.sync.dma_start(band01_sb[:, :], band01_dram[:, :])
            nc.sync.dma_start(i0_sb[:, :], i0_dram[:, :])

            # one-time zero inits for pad columns
            for s in range(NSET):
                nc.gpsimd.memset(kvb[s][:, 0:HALO], 0.0)
                nc.gpsimd.memset(kvb[s][:, HALO + L : W], 0.0)
                nc.gpsimd.memset(qb[s][:, L:W], 0.0)

            # ---- pools ----
            psA0 = ctx.enter_context(tc.tile_pool(name="psA0", bufs=2, space="PSUM"))
            psA1 = ctx.enter_context(tc.tile_pool(name="psA1", bufs=2, space="PSUM"))
            psB0 = ctx.enter_context(tc.tile_pool(name="psB0", bufs=1, space="PSUM"))
            psB1 = ctx.enter_context(tc.tile_pool(name="psB1", bufs=1, space="PSUM"))
            psC = ctx.enter_context(tc.tile_pool(name="psC", bufs=2, space="PSUM"))
            spP = ctx.enter_context(tc.tile_pool(name="spP", bufs=6))
            spV = ctx.enter_context(tc.tile_pool(name="spV", bufs=4))
            spR = ctx.enter_context(tc.tile_pool(name="spR", bufs=3))
            spS = ctx.enter_context(tc.tile_pool(name="spS", bufs=4))

            for pair in range(npair):
                s = pair % NSET
                # one DMA + one cast each for kv and q
                for ci in range(4):
                    c0, c1 = ci * (L // 4), (ci + 1) * (L // 4)
                    nc.sync.dma_start(kvf[s][:, c0:c1], kvs[pair, :, c0:c1])
                    nc.sync.dma_start(qf[s][:, c0:c1], q2[pair, :, c0:c1])
                H4 = L // 8
                for ci in range(8):
                    c0, c1 = ci * H4, (ci + 1) * H4
                    nc.gpsimd.tensor_copy(
                        kvb[s][:, HALO + c0 : HALO + c1], kvf[s][:, c0:c1]
                    )
                    nc.gpsimd.tensor_copy(qb[s][:, c0:c1], qf[s][:, c0:c1])
                # standalone weight loads let PE observe the cast completion
                # here, keeping later matmuls at <=2 sync waits (ISA limit)
                nc.tensor.ldweights(kvb[s][0:32, 0:128])
                nc.tensor.ldweights(qb[s][0:32, 0:128])

                for g in range(ngroups):
                    A = [
                        psA0.tile([128, TSTEP * G], F32, name="A0", tag="A0"),
                        psA1.tile([128, TSTEP * G], F32, name="A1", tag="A1"),
                    ]
                    Bp = [
                        psB0.tile([128, 32 * G], F32, name="B0", tag="B0"),
                        psB1.tile([128, 32 * G], F32, name="B1", tag="B1"),
                    ]
                    Cp = psC.tile([128, 66 * G], F32, name="Cp")
                    for l in range(G):
                        t0 = (G * g + l) * TSTEP
                        for bh in range(2):
                            kbase = 32 * bh       # k rows of this head
                            vbase = 64 + 32 * bh  # v rows of this head
                            nc.tensor.matmul(
                                A[bh][:, TSTEP * l : TSTEP * (l + 1)],
                                kvb[s][kbase : kbase + 32, t0 : t0 + SLAB],
                                qb[s][kbase : kbase + 32, t0 : t0 + TSTEP],
                                start=True,
                                stop=True,
                                tile_position=(kbase, 0),
                            )
                            nc.tensor.matmul(
                                Bp[bh][:, 32 * l : 32 * (l + 1)],
                                kvb[s][vbase : vbase + 32, t0 : t0 + SLAB],
                                i0_sb[vbase : vbase + 32, :],
                                start=True,
                                stop=True,
                                tile_position=(vbase, 0),
                            )
                    P = [
                        spP.tile([128, TSTEP * G], BF16, name="P0", tag="P0"),
                        spP.tile([128, TSTEP * G], BF16, name="P1", tag="P1"),
                    ]
                    # vT: [128, 2G slots of (32 v-cols + 1 ones-col)]
                    vT = spV.tile([128, 33 * 2 * G], BF16, name="vT")
                    nc.gpsimd.memset(vT[:, 32 :: 33], 1.0)
                    vt3 = vT[:, :].rearrange("p (s d) -> p s d", s=2 * G)[:, :, 0:32]
                    for bh in range(2):
                        nc.scalar.activation(
                            P[bh][:, :],
                            A[bh][:, :],
                            mybir.ActivationFunctionType.Exp,
                            bias=0.0,
                            scale=SCALE,
                        )
                        nc.vector.tensor_mul(P[bh][:, :], P[bh][:, :], band01_sb[:, :])
                        nc.vector.tensor_copy(
                            vt3[:, G * bh : G * (bh + 1), :],
                            Bp[bh][:, :].rearrange("p (s d) -> p s d", s=G),
                        )
                    for bh in range(2):
                        for l in range(G):
                            slot = G * bh + l
                            nc.tensor.matmul(
                                Cp[0:TSTEP, 33 * slot : 33 * (slot + 1)],
                                P[bh][:, TSTEP * l : TSTEP * (l + 1)],
                                vT[:, 33 * slot : 33 * (slot + 1)],
                                start=True,
                                stop=True,
                            )
                    r = spR.tile([128, 2 * G], F32, name="r")
                    nc.vector.reciprocal(r[0:TSTEP, :], Cp[0:TSTEP, 32 :: 33])
                    stage = spS.tile([128, 32 * 2 * G], F32, name="stage")
                    st_ap = stage[0:TSTEP, :].rearrange("p (s d) -> p s d", s=2 * G)
                    c_ap = Cp[0:TSTEP, :].rearrange("p (s d) -> p s d", s=2 * G)[:, :, 0:32]
                    r_b = r[0:TSTEP, :]
                    r_ap = bass.AP(
                        tensor=r_b.tensor,
                        offset=r_b.offset,
                        ap=[r_b.ap[0], [1, 2 * G], [0, 32]],
                    )
                    nc.vector.tensor_tensor(st_ap, c_ap, r_ap, op=mybir.AluOpType.mult)
                    # one contiguous output DMA per group (host reassembles)
                    nc.sync.dma_start(
                        out[pair, g, :, :], stage[0:TSTEP, :]
                    )
    nc.finalize()
    return nc


_CACHE = {}


def _get_program():
    if "nc" not in _CACHE:
        _CACHE["nc"] = _build_program()
    return _CACHE["nc"]


def make_in_maps(q, k, v):
    """Shard + pack FULL inputs into per-core input maps (host-side data
    movement only)."""
    q = np.ascontiguousarray(np.asarray(q), dtype=np.float32)
    k = np.ascontiguousarray(np.asarray(k), dtype=np.float32)
    v = np.ascontiguousarray(np.asarray(v), dtype=np.float32)
    qr = q.reshape(B * NHEAD, HD, L)
    kr = k.reshape(B * NHEAD, HD, L)
    vr = v.reshape(B * NHEAD, HD, L)

    in_maps = []
    for c in range(NCORES):
        base = c * BH_PER_CORE
        kvs = np.empty((NPAIR, 128, L), dtype=np.float32)
        q2 = np.empty((NPAIR, 64, L), dtype=np.float32)
        for p in range(NPAIR):
            i, j = base + 2 * p, base + 2 * p + 1
            kvs[p, 0:32] = kr[i]
            kvs[p, 32:64] = kr[j]
            kvs[p, 64:96] = vr[i]
            kvs[p, 96:128] = vr[j]
            q2[p, 0:32] = qr[i]
            q2[p, 32:64] = qr[j]
        in_maps.append({"kvs": kvs, "q2": q2})
    return in_maps


def assemble_output(results):
    """results: list of 8 per-core dicts with 'out'
    [NPAIR, NG, TSTEP, 2*G*HD]; scratch[p, g, t, (b, l, d)] holds token
    (G*g + l)*TSTEP + t of head-pair p, head b."""
    full = np.empty((B * NHEAD, L, HD), dtype=np.float32)
    for c in range(NCORES):
        sc = results[c]["out"].reshape(NPAIR, NG, TSTEP, 2, G, HD)
        # -> [pair, b, g, l, t, d] -> [bh, pos, d]
        sc = sc.transpose(0, 3, 1, 4, 2, 5).reshape(2 * NPAIR, NG * G * TSTEP, HD)
        full[c * BH_PER_CORE : (c + 1) * BH_PER_CORE] = sc[:, :L, :]
    full = full.reshape(B, NHEAD, L, HD).transpose(0, 2, 1, 3).reshape(B, L, D)
    return full.reshape(B, 1, L, D)


def kernel(q, k, v):
    from concourse.bass_utils import run_bass_kernel_spmd

    in_maps = make_in_maps(q, k, v)
    nc = _get_program()
    res = run_bass_kernel_spmd(nc, in_maps, core_ids=list(range(NCORES)))
    return assemble_output(res.results)


if __name__ == "__main__":
    rng = np.random.default_rng(0)
    q = rng.standard_normal((B, D, 1, L), dtype=np.float32)
    k = rng.standard_normal((B, D, 1, L), dtype=np.float32)
    v = rng.standard_normal((B, D, 1, L), dtype=np.float32)
    o = kernel(q=q, k=k, v=v)
    print("out", o.shape, o.dtype, float(np.abs(o).max()))


# revision 34
# speedup vs baseline: 1.0456x; 1.0154x over previous
"""DilateAttention (kernel=9, dilation=3, hd=32) on 8 NeuronCores via Bass/Tile.

Inputs  q,k,v: [4, 512, 1, 4096] f32  (B, d, 1, L); d = 16 heads x 32.
Output        [4, 1, 4096, 512] f32  (heads concatenated per token).

Math per (b, h): token n attends keys at n + 3*m - 12, m in 0..8 (zero-padded
outside [0, L)).  softmax over the 9 taps includes score-0 entries for
out-of-range taps (nn.Unfold zero-padding semantics).

Distribution: 64 (b,h) pairs -> 8 per core.  Host packs, per core, pairs of
heads into "kvs" [4, 128, L] (rows: k_i, k_j, v_i, v_j) and "q2" [4, 64, L]
(rows: q_i, q_j); pure slicing/stacking, all FLOPs happen on-device.

Per-core kernel (same SPMD program on all 8 cores, different data):
  Per head-pair: one DMA + one full-lane f32->bf16 cast for kv and for q.
  kvb [128, 4192] bf16 columns are shifted by HALO=12 with zero pads at both
  ends; qb [64, 4192] has zero pad columns at [L, W).
  For each 104-token tile t0 (key slab = positions [t0-12, t0+116)):
    mm1: S^T[u,t] = sum_d k[d, t0-12+u] * q[d, t0+t]      -> PSUM A
         (lhsT = kvb k-rows slab, rhs = qb block; 32-row contraction)
    mm2: vT[u,j]  = v[j, t0-12+u]                         -> PSUM B
         (lhsT = kvb v-rows slab, rhs = 32x32 identity)
    The two heads of a pair and mm1/mm2 sit on 4 distinct PE row-groups
    (partitions 0/32/64/96) so all four matmuls run concurrently.
    exp:  P^T = exp(S^T / sqrt(32))    ACT, PSUM->SBUF bf16
    band: P^T *= band01[u-t in {0,3,...,24}]   GPSIMD
    PV:   C[t, 0:32] = sum_u P^T[u,t] * vT[u,d]; a ones column appended to
          vT gives the softmax denominator in C[t, 32].
    normalize on DVE (reciprocal + broadcast multiply), DMA out.
  Tiles are processed in groups of G=4 sharing PSUM banks to amortize
  per-instruction overheads.
"""

import numpy as np

import concourse.bacc as bacc
import concourse.bass as bass
import concourse.mybir as mybir
from concourse.tile import TileContext

B, D, L = 4, 512, 4096
HD = 32
NHEAD = D // HD          # 16
NCORES = 8
BH_PER_CORE = (B * NHEAD) // NCORES   # 8
NPAIR = BH_PER_CORE // 2              # 4
HALO = 12                # dilation * (kernel-1) // 2
TSTEP = 104              # queries per tile = 128 - 2*HALO
SLAB = 128               # keys per tile
NT = (L + TSTEP - 1) // TSTEP         # 40 tiles per (b,h)
G = 4                    # tiles per PSUM group
NG = NT // G             # 10 groups
W = 4192                 # padded SBUF width (12 + 4096 + 12, rounded up)
SCALE = float(HD) ** -0.5

F32 = mybir.dt.float32
BF16 = mybir.dt.bfloat16


def _band01_np():
    # band01[u, t] = 1 iff key (t0-12+u) is a tap of query (t0+t):
    # u-t in {0, 3, ..., 24}.  t runs over the TSTEP valid queries per tile;
    # tiled Gx along t for the grouped/packed layout.
    u = np.arange(128)[:, None]
    t = np.arange(TSTEP)[None, :]
    d = u - t
    b = ((d >= 0) & (d <= 24) & (d % 3 == 0)).astype(np.float32)
    return np.tile(b, (1, G))


def _build_program(npair=NPAIR, ngroups=NG):
    import ml_dtypes

    nc = bacc.Bacc(None, target_bir_lowering=False)
    kvs = nc.dram_tensor("kvs", [NPAIR, 128, L], F32, kind="ExternalInput")
    q2 = nc.dram_tensor("q2", [NPAIR, 64, L], F32, kind="ExternalInput")
    out = nc.dram_tensor("out", [NPAIR, NG, TSTEP, 2 * G * HD], F32, kind="ExternalOutput")

    band01_dram = nc.inline_tensor(
        _band01_np().astype(ml_dtypes.bfloat16), name="band01"
    )
    # identities selecting the v rows (kvb rows 64-95 for head i, 96-127 for j)
    i0 = np.zeros((128, 32), dtype=ml_dtypes.bfloat16)
    for j in range(32):
        i0[64 + j, j] = 1.0
        i0[96 + j, j] = 1.0
    i0_dram = nc.inline_tensor(i0, name="i0ext")

    with TileContext(nc) as tc:
        from contextlib import ExitStack

        with ExitStack() as ctx:
            # ---- persistent tiles from a bufs=1 pool (ping-pong pairs) ----
            persist = ctx.enter_context(tc.tile_pool(name="persist", bufs=1))
            NSET = 3
            kvf = [
                persist.tile([128, L], F32, name=f"kvf{s}", tag=f"kvf{s}")
                for s in range(NSET)
            ]
            qf = [
                persist.tile([64, L], F32, name=f"qf{s}", tag=f"qf{s}")
                for s in range(NSET)
            ]
            kvb = [
                persist.tile([128, W], BF16, name=f"kvb{s}", tag=f"kvb{s}")
                for s in range(NSET)
            ]
            qb = [
                persist.tile([64, W], BF16, name=f"qb{s}", tag=f"qb{s}")
                for s in range(NSET)
            ]
            vTring = [
                persist.tile([128, 33 * 2 * G], BF16, name=f"vT{j}", tag=f"vT{j}")
                for j in range(4)
            ]
            band01_sb = persist.tile([128, TSTEP * G], BF16, name="band01_sb", tag="band01_sb")
            i0_sb = persist.tile([128, 32], BF16, name="i0_sb", tag="i0_sb")

            nc.sync.dma_start(band01_sb[:, :], band01_dram[:, :])
            nc.sync.dma_start(i0_sb[:, :], i0_dram[:, :])

            for j in range(4):
                nc.gpsimd.memset(vTring[j][:, 32 :: 33], 1.0)
            # one-time zero inits for pad columns
            for s in range(NSET):
                nc.gpsimd.memset(kvb[s][:, 0:HALO], 0.0)
                nc.gpsimd.memset(kvb[s][:, HALO + L : W], 0.0)
                nc.gpsimd.memset(qb[s][:, L:W], 0.0)

            # ---- pools ----
            psA0 = ctx.enter_context(tc.tile_pool(name="psA0", bufs=2, space="PSUM"))
            psA1 = ctx.enter_context(tc.tile_pool(name="psA1", bufs=2, space="PSUM"))
            psB0 = ctx.enter_context(tc.tile_pool(name="psB0", bufs=1, space="PSUM"))
            psB1 = ctx.enter_context(tc.tile_pool(name="psB1", bufs=1, space="PSUM"))
            psC = ctx.enter_context(tc.tile_pool(name="psC", bufs=2, space="PSUM"))
            spP = ctx.enter_context(tc.tile_pool(name="spP", bufs=6))
            spV = ctx.enter_context(tc.tile_pool(name="spV", bufs=4))
            spR = ctx.enter_context(tc.tile_pool(name="spR", bufs=3))
            spS = ctx.enter_context(tc.tile_pool(name="spS", bufs=4))

            for pair in range(npair):
                s = pair % NSET
                # one DMA + one cast each for kv and q
                for ci in range(4):
                    c0, c1 = ci * (L // 4), (ci + 1) * (L // 4)
                    nc.sync.dma_start(kvf[s][:, c0:c1], kvs[pair, :, c0:c1])
                    nc.sync.dma_start(qf[s][:, c0:c1], q2[pair, :, c0:c1])
                H4 = L // 8
                for ci in range(8):
                    c0, c1 = ci * H4, (ci + 1) * H4
                    nc.gpsimd.tensor_copy(
                        kvb[s][:, HALO + c0 : HALO + c1], kvf[s][:, c0:c1]
                    )
                    nc.gpsimd.tensor_copy(qb[s][:, c0:c1], qf[s][:, c0:c1])
                # standalone weight loads let PE observe the cast completion
                # here, keeping later matmuls at <=2 sync waits (ISA limit)
                nc.tensor.ldweights(kvb[s][0:32, 0:128])
                nc.tensor.ldweights(qb[s][0:32, 0:128])

                for g in range(ngroups):
                    A = [
                        psA0.tile([128, TSTEP * G], F32, name="A0", tag="A0"),
                        psA1.tile([128, TSTEP * G], F32, name="A1", tag="A1"),
                    ]
                    Bp = [
                        psB0.tile([128, 32 * G], F32, name="B0", tag="B0"),
                        psB1.tile([128, 32 * G], F32, name="B1", tag="B1"),
                    ]
                    Cp = psC.tile([128, 66 * G], F32, name="Cp")
                    for l in range(G):
                        t0 = (G * g + l) * TSTEP
                        for bh in range(2):
                            kbase = 32 * bh       # k rows of this head
                            vbase = 64 + 32 * bh  # v rows of this head
                            nc.tensor.matmul(
                                A[bh][:, TSTEP * l : TSTEP * (l + 1)],
                                kvb[s][kbase : kbase + 32, t0 : t0 + SLAB],
                                qb[s][kbase : kbase + 32, t0 : t0 + TSTEP],
                                start=True,
                                stop=True,
                                tile_position=(kbase, 0),
                            )
                            nc.tensor.matmul(
                                Bp[bh][:, 32 * l : 32 * (l + 1)],
                                kvb[s][vbase : vbase + 32, t0 : t0 + SLAB],
                                i0_sb[vbase : vbase + 32, :],
                                start=True,
                                stop=True,
                                tile_position=(vbase, 0),
                            )
                    P = [
                        spP.tile([128, TSTEP * G], BF16, name="P0", tag="P0"),
                        spP.tile([128, TSTEP * G], BF16, name="P1", tag="P1"),
                    ]
                    # vT: [128, 2G slots of (32 v-cols + 1 ones-col)];
                    # persistent ring so the ones columns are written once
                    vT = vTring[(pair * ngroups + g) % 4]
                    vt3 = vT[:, :].rearrange("p (s d) -> p s d", s=2 * G)[:, :, 0:32]
                    for bh in range(2):
                        nc.scalar.activation(
                            P[bh][:, :],
                            A[bh][:, :],
                            mybir.ActivationFunctionType.Exp,
                            bias=0.0,
                            scale=SCALE,
                        )
                        nc.vector.tensor_mul(P[bh][:, :], P[bh][:, :], band01_sb[:, :])
                        nc.vector.tensor_copy(
                            vt3[:, G * bh : G * (bh + 1), :],
                            Bp[bh][:, :].rearrange("p (s d) -> p s d", s=G),
                        )
                    for bh in range(2):
                        for l in range(G):
                            slot = G * bh + l
                            nc.tensor.matmul(
                                Cp[0:TSTEP, 33 * slot : 33 * (slot + 1)],
                                P[bh][:, TSTEP * l : TSTEP * (l + 1)],
                                vT[:, 33 * slot : 33 * (slot + 1)],
                                start=True,
                                stop=True,
                            )
                    r = spR.tile([128, 2 * G], F32, name="r")
                    nc.vector.reciprocal(r[0:TSTEP, :], Cp[0:TSTEP, 32 :: 33])
                    stage = spS.tile([128, 32 * 2 * G], F32, name="stage")
                    st_ap = stage[0:TSTEP, :].rearrange("p (s d) -> p s d", s=2 * G)
                    c_ap = Cp[0:TSTEP, :].rearrange("p (s d) -> p s d", s=2 * G)[:, :, 0:32]
                    r_b = r[0:TSTEP, :]
                    r_ap = bass.AP(
                        tensor=r_b.tensor,
                        offset=r_b.offset,
                        ap=[r_b.ap[0], [1, 2 * G], [0, 32]],
                    )
                    nc.vector.tensor_tensor(st_ap, c_ap, r_ap, op=mybir.AluOpType.mult)
                    # one contiguous output DMA per group (host reassembles)
                    nc.sync.dma_start(
                        out[pair, g, :, :], stage[0:TSTEP, :]
                    )
    nc.finalize()
    return nc


_CACHE = {}


def _get_program():
    if "nc" not in _CACHE:
        _CACHE["nc"] = _build_program()
    return _CACHE["nc"]


def make_in_maps(q, k, v):
    """Shard + pack FULL inputs into per-core input maps (host-side data
    movement only)."""
    q = np.ascontiguousarray(np.asarray(q), dtype=np.float32)
    k = np.ascontiguousarray(np.asarray(k), dtype=np.float32)
    v = np.ascontiguousarray(np.asarray(v), dtype=np.float32)
    qr = q.reshape(B * NHEAD, HD, L)
    kr = k.reshape(B * NHEAD, HD, L)
    vr = v.reshape(B * NHEAD, HD, L)

    in_maps = []
    for c in range(NCORES):
        base = c * BH_PER_CORE
        kvs = np.empty((NPAIR, 128, L), dtype=np.float32)
        q2 = np.empty((NPAIR, 64, L), dtype=np.float32)
        for p in range(NPAIR):
            i, j = base + 2 * p, base + 2 * p + 1
            kvs[p, 0:32] = kr[i]
            kvs[p, 32:64] = kr[j]
            kvs[p, 64:96] = vr[i]
            kvs[p, 96:128] = vr[j]
            q2[p, 0:32] = qr[i]
            q2[p, 32:64] = qr[j]
        in_maps.append({"kvs": kvs, "q2": q2})
    return in_maps


def assemble_output(results):
    """results: list of 8 per-core dicts with 'out'
    [NPAIR, NG, TSTEP, 2*G*HD]; scratch[p, g, t, (b, l, d)] holds token
    (G*g + l)*TSTEP + t of head-pair p, head b."""
    full = np.empty((B * NHEAD, L, HD), dtype=np.float32)
    for c in range(NCORES):
        sc = results[c]["out"].reshape(NPAIR, NG, TSTEP, 2, G, HD)
        # -> [pair, b, g, l, t, d] -> [bh, pos, d]
        sc = sc.transpose(0, 3, 1, 4, 2, 5).reshape(2 * NPAIR, NG * G * TSTEP, HD)
        full[c * BH_PER_CORE : (c + 1) * BH_PER_CORE] = sc[:, :L, :]
    full = full.reshape(B, NHEAD, L, HD).transpose(0, 2, 1, 3).reshape(B, L, D)
    return full.reshape(B, 1, L, D)


def kernel(q, k, v):
    from concourse.bass_utils import run_bass_kernel_spmd

    in_maps = make_in_maps(q, k, v)
    nc = _get_program()
    res = run_bass_kernel_spmd(nc, in_maps, core_ids=list(range(NCORES)))
    return assemble_output(res.results)


if __name__ == "__main__":
    rng = np.random.default_rng(0)
    q = rng.standard_normal((B, D, 1, L), dtype=np.float32)
    k = rng.standard_normal((B, D, 1, L), dtype=np.float32)
    v = rng.standard_normal((B, D, 1, L), dtype=np.float32)
    o = kernel(q=q, k=k, v=v)
    print("out", o.shape, o.dtype, float(np.abs(o).max()))


# revision 43
# speedup vs baseline: 1.0829x; 1.0356x over previous
"""DilateAttention (kernel=9, dilation=3, hd=32) on 8 NeuronCores via Bass/Tile.

Inputs  q,k,v: [4, 512, 1, 4096] f32  (B, d, 1, L); d = 16 heads x 32.
Output        [4, 1, 4096, 512] f32  (heads concatenated per token).

Math per (b, h): token n attends keys at n + 3*m - 12, m in 0..8 (zero-padded
outside [0, L)).  softmax over the 9 taps includes score-0 entries for
out-of-range taps (nn.Unfold zero-padding semantics).

Distribution: 64 (b,h) pairs -> 8 per core.  Host packs, per core, pairs of
heads into "kvs" [4, 128, L] (rows: k_i, k_j, v_i, v_j) and "q2" [4, 64, L]
(rows: q_i, q_j); pure slicing/stacking, all FLOPs happen on-device.

Per-core kernel (same SPMD program on all 8 cores, different data):
  Per head-pair: one DMA + one full-lane f32->bf16 cast for kv and for q.
  kvb [128, 4192] bf16 columns are shifted by HALO=12 with zero pads at both
  ends; qb [64, 4192] has zero pad columns at [L, W).
  For each 104-token tile t0 (key slab = positions [t0-12, t0+116)):
    mm1: S^T[u,t] = sum_d k[d, t0-12+u] * q[d, t0+t]      -> PSUM A
         (lhsT = kvb k-rows slab, rhs = qb block; 32-row contraction)
    mm2: vT[u,j]  = v[j, t0-12+u]                         -> PSUM B
         (lhsT = kvb v-rows slab, rhs = 32x32 identity)
    The two heads of a pair and mm1/mm2 sit on 4 distinct PE row-groups
    (partitions 0/32/64/96) so all four matmuls run concurrently.
    exp:  P^T = exp(S^T / sqrt(32))    ACT, PSUM->SBUF bf16
    band: P^T *= band01[u-t in {0,3,...,24}]   GPSIMD
    PV:   C[t, 0:32] = sum_u P^T[u,t] * vT[u,d]; a ones column appended to
          vT gives the softmax denominator in C[t, 32].
    normalize on DVE (reciprocal + broadcast multiply), DMA out.
  Tiles are processed in groups of G=4 sharing PSUM banks to amortize
  per-instruction overheads.
"""

import numpy as np

import concourse.bacc as bacc
import concourse.bass as bass
import concourse.mybir as mybir
from concourse.tile import TileContext

B, D, L = 4, 512, 4096
HD = 32
NHEAD = D // HD          # 16
NCORES = 8
BH_PER_CORE = (B * NHEAD) // NCORES   # 8
NPAIR = BH_PER_CORE // 2              # 4
HALO = 12                # dilation * (kernel-1) // 2
TSTEP = 104              # queries per tile = 128 - 2*HALO
SLAB = 128               # keys per tile
NT = (L + TSTEP - 1) // TSTEP         # 40 tiles per (b,h)
G = 4                    # tiles per PSUM group
NG = NT // G             # 10 groups
W = 4192                 # padded SBUF width (12 + 4096 + 12, rounded up)
SCALE = float(HD) ** -0.5

F32 = mybir.dt.float32
BF16 = mybir.dt.bfloat16


def _band01_np():
    # band01[u, t] = 1 iff key (t0-12+u) is a tap of query (t0+t):
    # u-t in {0, 3, ..., 24}.  t runs over the TSTEP valid queries per tile;
    # tiled Gx along t for the grouped/packed layout.
    u = np.arange(128)[:, None]
    t = np.arange(TSTEP)[None, :]
    d = u - t
    b = ((d >= 0) & (d <= 24) & (d % 3 == 0)).astype(np.float32)
    return np.tile(b, (1, G))


def _build_program(npair=NPAIR, ngroups=NG):
    import ml_dtypes

    nc = bacc.Bacc(None, target_bir_lowering=False)
    kvs = nc.dram_tensor("kvs", [NPAIR, 128, L], F32, kind="ExternalInput")
    q2 = nc.dram_tensor("q2", [NPAIR, 64, L], F32, kind="ExternalInput")
    out = nc.dram_tensor("out", [NPAIR, NG, TSTEP, 2 * G * HD], F32, kind="ExternalOutput")

    band01_dram = nc.inline_tensor(
        _band01_np().astype(ml_dtypes.bfloat16), name="band01"
    )
    # identities selecting the v rows (kvb rows 64-95 for head i, 96-127 for j)
    i0 = np.zeros((128, 32), dtype=ml_dtypes.bfloat16)
    for j in range(32):
        i0[64 + j, j] = 1.0
        i0[96 + j, j] = 1.0
    i0_dram = nc.inline_tensor(i0, name="i0ext")

    with TileContext(nc) as tc:
        from contextlib import ExitStack

        with ExitStack() as ctx:
            # ---- persistent tiles from a bufs=1 pool (ping-pong pairs) ----
            persist = ctx.enter_context(tc.tile_pool(name="persist", bufs=1))
            NSET = 3
            kvf = [
                persist.tile([128, L], F32, name=f"kvf{s}", tag=f"kvf{s}")
                for s in range(NSET)
            ]
            qf = [
                persist.tile([64, L], F32, name=f"qf{s}", tag=f"qf{s}")
                for s in range(NSET)
            ]
            kvb = [
                persist.tile([128, W], BF16, name=f"kvb{s}", tag=f"kvb{s}")
                for s in range(NSET)
            ]
            qb = [
                persist.tile([64, W], BF16, name=f"qb{s}", tag=f"qb{s}")
                for s in range(NSET)
            ]
            vTring = [
                persist.tile([128, 33 * 2 * G], BF16, name=f"vT{j}", tag=f"vT{j}")
                for j in range(4)
            ]
            band01_sb = persist.tile([128, TSTEP * G], BF16, name="band01_sb", tag="band01_sb")
            i0_sb = persist.tile([128, 32], BF16, name="i0_sb", tag="i0_sb")

            nc.sync.dma_start(band01_sb[:, :], band01_dram[:, :])
            nc.sync.dma_start(i0_sb[:, :], i0_dram[:, :])

            for j in range(4):
                nc.gpsimd.memset(vTring[j][:, 32 :: 33], 1.0)
            # one-time zero inits for pad columns
            for s in range(NSET):
                nc.gpsimd.memset(kvb[s][:, 0:HALO], 0.0)
                nc.gpsimd.memset(kvb[s][:, HALO + L : W], 0.0)
                nc.gpsimd.memset(qb[s][:, L:W], 0.0)

            # ---- pools ----
            psA0 = ctx.enter_context(tc.tile_pool(name="psA0", bufs=1, space="PSUM"))
            psA1 = ctx.enter_context(tc.tile_pool(name="psA1", bufs=2, space="PSUM"))
            psB0 = ctx.enter_context(tc.tile_pool(name="psB0", bufs=2, space="PSUM"))
            psB1 = ctx.enter_context(tc.tile_pool(name="psB1", bufs=2, space="PSUM"))
            psC = ctx.enter_context(tc.tile_pool(name="psC", bufs=1, space="PSUM"))
            spP = ctx.enter_context(tc.tile_pool(name="spP", bufs=8))
            spV = ctx.enter_context(tc.tile_pool(name="spV", bufs=4))
            spR = ctx.enter_context(tc.tile_pool(name="spR", bufs=4))
            spS = ctx.enter_context(tc.tile_pool(name="spS", bufs=4))

            for pair in range(npair):
                s = pair % NSET
                # one DMA + one cast each for kv and q
                for ci in range(4):
                    c0, c1 = ci * (L // 4), (ci + 1) * (L // 4)
                    nc.sync.dma_start(kvf[s][:, c0:c1], kvs[pair, :, c0:c1])
                    nc.sync.dma_start(qf[s][:, c0:c1], q2[pair, :, c0:c1])
                H4 = L // 8
                for ci in range(8):
                    c0, c1 = ci * H4, (ci + 1) * H4
                    nc.gpsimd.tensor_copy(
                        kvb[s][:, HALO + c0 : HALO + c1], kvf[s][:, c0:c1]
                    )
                    nc.gpsimd.tensor_copy(qb[s][:, c0:c1], qf[s][:, c0:c1])
                # standalone weight loads let PE observe the cast completion
                # here, keeping later matmuls at <=2 sync waits (ISA limit)
                nc.tensor.ldweights(kvb[s][0:32, 0:128])
                nc.tensor.ldweights(qb[s][0:32, 0:128])

                for g in range(ngroups):
                    A = [
                        psA0.tile([128, TSTEP * G], F32, name="A0", tag="A0"),
                        psA1.tile([128, TSTEP * G], F32, name="A1", tag="A1"),
                    ]
                    Bp = [
                        psB0.tile([128, 32 * G], F32, name="B0", tag="B0"),
                        psB1.tile([128, 32 * G], F32, name="B1", tag="B1"),
                    ]
                    Cp = psC.tile([128, 66 * G], F32, name="Cp")
                    for l in range(G):
                        t0 = (G * g + l) * TSTEP
                        for bh in range(2):
                            kbase = 32 * bh       # k rows of this head
                            vbase = 64 + 32 * bh  # v rows of this head
                            nc.tensor.matmul(
                                A[bh][:, TSTEP * l : TSTEP * (l + 1)],
                                kvb[s][kbase : kbase + 32, t0 : t0 + SLAB],
                                qb[s][kbase : kbase + 32, t0 : t0 + TSTEP],
                                start=True,
                                stop=True,
                                tile_position=(kbase, 0),
                            )
                            nc.tensor.matmul(
                                Bp[bh][:, 32 * l : 32 * (l + 1)],
                                kvb[s][vbase : vbase + 32, t0 : t0 + SLAB],
                                i0_sb[vbase : vbase + 32, :],
                                start=True,
                                stop=True,
                                tile_position=(vbase, 0),
                            )
                    P = [
                        spP.tile([128, TSTEP * G], BF16, name="P0", tag="P0"),
                        spP.tile([128, TSTEP * G], BF16, name="P1", tag="P1"),
                    ]
                    # vT: [128, 2G slots of (32 v-cols + 1 ones-col)];
                    # persistent ring so the ones columns are written once
                    vT = vTring[(pair * ngroups + g) % 4]
                    vt3 = vT[:, :].rearrange("p (s d) -> p s d", s=2 * G)[:, :, 0:32]
                    for bh in range(2):
                        nc.scalar.activation(
                            P[bh][:, :],
                            A[bh][:, :],
                            mybir.ActivationFunctionType.Exp,
                            bias=0.0,
                            scale=SCALE,
                        )
                        nc.vector.tensor_mul(P[bh][:, :], P[bh][:, :], band01_sb[:, :])
                        nc.vector.tensor_copy(
                            vt3[:, G * bh : G * (bh + 1), :],
                            Bp[bh][:, :].rearrange("p (s d) -> p s d", s=G),
                        )
                    for bh in range(2):
                        for l in range(G):
                            slot = G * bh + l
                            nc.tensor.matmul(
                                Cp[0:TSTEP, 33 * slot : 33 * (slot + 1)],
                                P[bh][:, TSTEP * l : TSTEP * (l + 1)],
                                vT[:, 33 * slot : 33 * (slot + 1)],
                                start=True,
                                stop=True,
                            )
                    r = spR.tile([128, 2 * G], F32, name="r")
                    nc.vector.reciprocal(r[0:TSTEP, :], Cp[0:TSTEP, 32 :: 33])
                    stage = spS.tile([128, 32 * 2 * G], F32, name="stage")
                    st_ap = stage[0:TSTEP, :].rearrange("p (s d) -> p s d", s=2 * G)
                    c_ap = Cp[0:TSTEP, :].rearrange("p (s d) -> p s d", s=2 * G)[:, :, 0:32]
                    r_b = r[0:TSTEP, :]
                    r_ap = bass.AP(
                        tensor=r_b.tensor,
                        offset=r_b.offset,
                        ap=[r_b.ap[0], [1, 2 * G], [0, 32]],
                    )
                    nc.vector.tensor_tensor(st_ap, c_ap, r_ap, op=mybir.AluOpType.mult)
                    # one contiguous output DMA per group (host reassembles)
                    nc.sync.dma_start(
                        out[pair, g, :, :], stage[0:TSTEP, :]
                    )
    nc.finalize()
    return nc


_CACHE = {}


def _get_program():
    if "nc" not in _CACHE:
        _CACHE["nc"] = _build_program()
    return _CACHE["nc"]


def make_in_maps(q, k, v):
    """Shard + pack FULL inputs into per-core input maps (host-side data
    movement only)."""
    q = np.ascontiguousarray(np.asarray(q), dtype=np.float32)
    k = np.ascontiguousarray(np.asarray(k), dtype=np.float32)
    v = np.ascontiguousarray(np.asarray(v), dtype=np.float32)
    qr = q.reshape(B * NHEAD, HD, L)
    kr = k.reshape(B * NHEAD, HD, L)
    vr = v.reshape(B * NHEAD, HD, L)

    in_maps = []
    for c in range(NCORES):
        base = c * BH_PER_CORE
        kvs = np.empty((NPAIR, 128, L), dtype=np.float32)
        q2 = np.empty((NPAIR, 64, L), dtype=np.float32)
        for p in range(NPAIR):
            i, j = base + 2 * p, base + 2 * p + 1
            kvs[p, 0:32] = kr[i]
            kvs[p, 32:64] = kr[j]
            kvs[p, 64:96] = vr[i]
            kvs[p, 96:128] = vr[j]
            q2[p, 0:32] = qr[i]
            q2[p, 32:64] = qr[j]
        in_maps.append({"kvs": kvs, "q2": q2})
    return in_maps


def assemble_output(results):
    """results: list of 8 per-core dicts with 'out'
    [NPAIR, NG, TSTEP, 2*G*HD]; scratch[p, g, t, (b, l, d)] holds token
    (G*g + l)*TSTEP + t of head-pair p, head b."""
    full = np.empty((B * NHEAD, L, HD), dtype=np.float32)
    for c in range(NCORES):
        sc = results[c]["out"].reshape(NPAIR, NG, TSTEP, 2, G, HD)
        # -> [pair, b, g, l, t, d] -> [bh, pos, d]
        sc = sc.transpose(0, 3, 1, 4, 2, 5).reshape(2 * NPAIR, NG * G * TSTEP, HD)
        full[c * BH_PER_CORE : (c + 1) * BH_PER_CORE] = sc[:, :L, :]
    full = full.reshape(B, NHEAD, L, HD).transpose(0, 2, 1, 3).reshape(B, L, D)
    return full.reshape(B, 1, L, D)


def kernel(q, k, v):
    from concourse.bass_utils import run_bass_kernel_spmd

    in_maps = make_in_maps(q, k, v)
    nc = _get_program()
    res = run_bass_kernel_spmd(nc, in_maps, core_ids=list(range(NCORES)))
    return assemble_output(res.results)


if __name__ == "__main__":
    rng = np.random.default_rng(0)
    q = rng.standard_normal((B, D, 1, L), dtype=np.float32)
    k = rng.standard_normal((B, D, 1, L), dtype=np.float32)
    v = rng.standard_normal((B, D, 1, L), dtype=np.float32)
    o = kernel(q=q, k=k, v=v)
    print("out", o.shape, o.dtype, float(np.abs(o).max()))
